# revision 2
# baseline (speedup 1.0000x reference)
"""Multi-head attention (B=2, L=2048, D=2048, H=16, causal + RoPE) on 8 TRN2 cores.

Sharding: tensor-parallel over heads. Core c owns heads {2c, 2c+1}:
  - wq/wk/wv column slices [D, 256], wo row slice [256, D]
  - each core computes a partial output y_c = att_c @ wo_c (full shape, bf16)
  - host reduces: y = sum_c y_c  (the "all-reduce" of the output projection)

v2 design (all-bf16 operands, fp32 PSUM accumulation):
  1. Projections: QT/KT = (w^T x^T) in transposed layout [head_dim, tok],
     V in natural layout [tok, head_dim]; all matmul operands bf16 (1 cyc/row).
     RoPE applied to QT/KT rows 0:64 per head with head-duplicated cc/ss
     tables so each DVE op covers both heads ([64, 2, 512], bf16 2x mode).
  2. Attention per (batch, head): causal, no-max-subtraction softmax.
     Per j-block of 128 keys against a 512-query i-tile:
       ST = K_j^T Q_i   (PE)     -- diagonal blocks narrowed to valid columns
       ET = exp(ST)     (ACT, bf16 out) ; diagonal strip masked via one
                                  [128,128] tril mult (DVE, bf16 2x)
       R  += ones^T ET  (PE)     -- ones stationary is [128,128] so R PSUM
                                    holds denominators broadcast to all
                                    partitions: no transpose bounce needed
       AV += V_j^T ET   (PE)
     Then att[:, h, i-tile] = AV * reciprocal(R) -- two DVE ops, fused
     normalize + PSUM->SBUF copy.
  3. Output: y[tok,:] = sum_h att_h^T wo_h ; both heads accumulate into one
     PSUM bank; PSUM->SBUF copies alternate ACT/DVE into a [128, 4, 512]
     bf16 stage written out as one [128, 2048] DMA per 128-token chunk.
     Emissions are interleaved ("pending" queue) into later attention
     j-blocks and the next batch's projection chains so the PE never idles
     through an emission-only phase.
"""

import glob
import os


def _ensure_env():
    # walrus_driver (neuronx-cc) must be on PATH for client-side NEFF compile.
    if not any("-b16-bazel-" in p for p in os.environ.get("PATH", "").split(":")):
        cands = sorted(glob.glob("/nix/store/*-b16-bazel-*/bin"))
        for c in cands:
            if os.path.exists(os.path.join(c, "neuronx-cc")) or glob.glob(
                os.path.join(c, "*walrus*")
            ):
                os.environ["PATH"] = c + ":" + os.environ["PATH"]
                break
        else:
            if cands:
                os.environ["PATH"] = cands[-1] + ":" + os.environ["PATH"]


_ensure_env()
os.environ.setdefault("JAX_COMPILATION_CACHE_DIR", "/tmp/jax_comp_cache")
os.environ.setdefault("JAX_PERSISTENT_CACHE_MIN_COMPILE_TIME_SECS", "1")
os.environ.setdefault("JAX_PERSISTENT_CACHE_MIN_ENTRY_SIZE_BYTES", "0")

import numpy as np  # noqa: E402

import concourse.bass as bass  # noqa: E402
import concourse.mybir as mybir  # noqa: E402
import concourse.tile as tile  # noqa: E402
from concourse import bacc  # noqa: E402
from concourse.bass_utils import run_bass_kernel_spmd  # noqa: E402

NCORES = 8
B, L, D = 2, 2048, 2048
H = 16
HD = 128            # head dim
HPC = H // NCORES   # heads per core
DQ = HPC * HD       # 256: per-core projection width
ROPE = 64           # RoPE dims per head
F32 = mybir.dt.float32
BF16 = mybir.dt.bfloat16

NTT = L // 512      # 4 token tiles (512) for projections
NI = L // 512       # 4 i-tiles per attention instance
NJ = L // 128       # 16 j-blocks


def build_nc():
    nc = bacc.Bacc(
        "TRN2", target_bir_lowering=False, debug=False, num_devices=NCORES
    )
    xt = nc.dram_tensor("xt", [B, D, L], BF16, kind="ExternalInput").ap()
    wq = nc.dram_tensor("wq", [D, DQ], BF16, kind="ExternalInput").ap()
    wk = nc.dram_tensor("wk", [D, DQ], BF16, kind="ExternalInput").ap()
    wv = nc.dram_tensor("wv", [D, DQ], BF16, kind="ExternalInput").ap()
    wo = nc.dram_tensor("wo", [DQ, D], BF16, kind="ExternalInput").ap()
    # cc rows = [cos;cos], ss rows = [-sin;+sin], duplicated per head so one
    # DVE op can cover both heads' rope rows: [ROPE, HPC, L]
    cc = nc.dram_tensor("cc", [ROPE, HPC, L], BF16, kind="ExternalInput").ap()
    ss = nc.dram_tensor("ss", [ROPE, HPC, L], BF16, kind="ExternalInput").ap()
    # strip mask for diagonal j-blocks: m1[j, x] = 1.0 if x >= j else 0
    m1 = nc.dram_tensor("m1", [128, 128], BF16, kind="ExternalInput").ap()
    y = nc.dram_tensor("y", [B, L, D], BF16, kind="ExternalOutput").ap()

    with tile.TileContext(nc) as tc:
        with (
            tc.tile_pool(name="consts", bufs=1) as consts,
            tc.tile_pool(name="wpool", bufs=1) as wpool,
            tc.tile_pool(name="qkv", bufs=1) as qkv,
            tc.tile_pool(name="xc", bufs=2) as xcpool,
            tc.tile_pool(name="et", bufs=6) as etpool,
            tc.tile_pool(name="rope", bufs=4) as ropepool,
            tc.tile_pool(name="rec", bufs=2) as recpool,
            tc.tile_pool(name="ysb", bufs=4) as ypool,
            tc.tile_pool(name="pA", bufs=2, space="PSUM") as pA,
            tc.tile_pool(name="pST", bufs=3, space="PSUM") as pST,
            tc.tile_pool(name="pAV", bufs=2, space="PSUM") as pAV,
            tc.tile_pool(name="pR", bufs=1, space="PSUM") as pR,
        ):
            # ---- constants / weights ----
            ones = consts.tile([128, 128], BF16)
            nc.vector.memset(ones, 1.0)
            # warm the PE p-state during the startup DMA wait: ~80 tiny
            # matmuls keep the tensor engine continuously busy so the first
            # real chains run at full clock (ramp needs 3us of busy)
            pwarm = pR.tile([128, 64], F32, tag="pR", name="pwarm")
            for _ in range(60):
                nc.tensor.matmul(
                    pwarm[0:1, :], ones[:, 0:1], ones[:, 0:64],
                    start=True, stop=True,
                )
            m1_sb = consts.tile([128, 128], BF16)
            cc_sb = consts.tile([ROPE, HPC, L], BF16)
            ss_sb = consts.tile([ROPE, HPC, L], BF16)
            xc_pre = xcpool.tile([128, 16, 512], BF16, tag="xc")
            wq_sb = wpool.tile([128, 16, DQ], BF16)
            wk_sb = wpool.tile([128, 16, DQ], BF16)
            wv_sb = wpool.tile([128, 16, DQ], BF16)
            wo_sb = wpool.tile([128, HPC, D], BF16)

            def _w_oct(w_dram, w_sb, oct_):
                nc.sync.dma_start(
                    out=w_sb[:, 2 * oct_ : 2 * oct_ + 2, :],
                    in_=w_dram[256 * oct_ : 256 * oct_ + 256, :].rearrange(
                        "(c p) o -> p c o", p=128
                    ),
                )

            # DMA order = first-use order: the first q-projection chain needs
            # xc oct_c + wq oct_c in sequence, so interleave them; wk/wv feed
            # the following chains; rope tables before the first rope; wo and
            # the mask strip are only needed tens of us in.
            for oct_ in range(8):
                nc.sync.dma_start(
                    out=xc_pre[:, 2 * oct_ : 2 * oct_ + 2, :],
                    in_=xt[0, 256 * oct_ : 256 * oct_ + 256, 0:512].rearrange(
                        "(c p) t -> p c t", p=128
                    ),
                )
                _w_oct(wq, wq_sb, oct_)
            for w_dram, w_sb in ((wk, wk_sb), (wv, wv_sb)):
                for qq in range(4):
                    nc.sync.dma_start(
                        out=w_sb[:, 4 * qq : 4 * qq + 4, :],
                        in_=w_dram[
                            512 * qq : 512 * qq + 512, :
                        ].rearrange("(c p) o -> p c o", p=128),
                    )

            qt_sb = qkv.tile([128, HPC, L], BF16)   # [d, h, tok]
            kt_sb = qkv.tile([128, HPC, L], BF16)
            v_sb = qkv.tile([128, NJ, DQ], BF16)    # [tok_in_blk, jblk, hd]
            att_sb = qkv.tile([128, HPC, L], BF16)  # [hd, h, tok] normalized

            # ---- y emission machinery ----
            pending = []          # (b, t2) chunks with dd sub-emissions
            ystage = {}           # t2 -> stage tile
            ecount = [0]          # emission counter for ACT/DVE alternation

            def emit_y(b_, t2, dd, pool=None, tag="pA", eng="alt", split_dma=False):
                pool = pool or pA
                p = pool.tile([128, 512], F32, tag=tag, name=f"yp_{b_}_{t2}_{dd}")
                nc.tensor.matmul(
                    p,
                    att_sb[:, 0, 128 * t2 : 128 * t2 + 128],
                    wo_sb[:, 0, 512 * dd : 512 * dd + 512],
                    start=True,
                    stop=False,
                )
                nc.tensor.matmul(
                    p,
                    att_sb[:, 1, 128 * t2 : 128 * t2 + 128],
                    wo_sb[:, 1, 512 * dd : 512 * dd + 512],
                    start=False,
                    stop=True,
                )
                if dd == 0:
                    ystage[t2] = ypool.tile(
                        [128, 4, 512], BF16, tag="ysb", name=f"yst_{b_}_{t2}"
                    )
                dst = ystage[t2][:, dd, :]
                # ACT is the exp critical path during attention: route those
                # drains' copies to DVE; alternate engines elsewhere
                if eng == "dve" or (eng == "alt" and ecount[0] % 2 == 1):
                    nc.vector.tensor_copy(dst, p)
                else:
                    nc.scalar.activation(
                        dst, p, mybir.ActivationFunctionType.Copy
                    )
                ecount[0] += 1
                if split_dma:
                    qeng = (nc.gpsimd, nc.scalar, nc.gpsimd, nc.sync)[dd]
                    qeng.dma_start(
                        out=y[
                            b_,
                            128 * t2 : 128 * t2 + 128,
                            512 * dd : 512 * dd + 512,
                        ],
                        in_=dst,
                    )
                    if dd == 3:
                        ystage.pop(t2)
                elif dd == 3:
                    nc.gpsimd.dma_start(
                        out=y[b_, 128 * t2 : 128 * t2 + 128, :],
                        in_=ystage.pop(t2),
                    )

            def drain(n=1, eng="alt", pool=None, tag="pA"):
                for _ in range(n):
                    if pending:
                        emit_y(*pending.pop(0), eng=eng, pool=pool, tag=tag)

            xc_tiles = {(0, 0): xc_pre}

            def xc_load(b_, tt_):
                # prefetch: issue the x-chunk DMAs one tile ahead of use
                if (b_, tt_) in xc_tiles or b_ >= B:
                    return
                xct = xcpool.tile(
                    [128, 16, 512], BF16, tag="xc", name=f"xc_{b_}_{tt_}"
                )
                for qq in range(4):
                    nc.sync.dma_start(
                        out=xct[:, 4 * qq : 4 * qq + 4, :],
                        in_=xt[
                            b_,
                            512 * qq : 512 * qq + 512,
                            512 * tt_ : 512 * tt_ + 512,
                        ].rearrange("(c p) t -> p c t", p=128),
                    )
                xc_tiles[(b_, tt_)] = xct

            deferred_v = []

            for b in range(B):
                # ---------- projections ----------
                for tt in range(NTT):
                    xc_load(b, tt)
                    xc = xc_tiles.pop((b, tt))
                    if tt + 1 < NTT:
                        xc_load(b, tt + 1)
                    else:
                        xc_load(b + 1, 0)
                    if b == 0 and tt == 0:
                        # tables/masks/wo are needed late; issue after the
                        # tt1 x prefetch so that lands first
                        nc.sync.dma_start(out=cc_sb, in_=cc)
                        nc.sync.dma_start(out=ss_sb, in_=ss)
                        nc.sync.dma_start(out=m1_sb, in_=m1)
                        for hh in range(HPC):
                            nc.sync.dma_start(
                                out=wo_sb[:, hh, :],
                                in_=wo[HD * hh : HD * hh + HD, :],
                            )
                    # QT / KT rows (transposed layout); RoPE applied below
                    for w_sb, out_sb in (
                        (wq_sb, qt_sb),
                        (wk_sb, kt_sb),
                    ):
                        for rt in range(HPC):
                            pp = pA.tile([128, 512], F32, tag="pA")
                            for c in range(16):
                                nc.tensor.matmul(
                                    pp,
                                    w_sb[:, c, 128 * rt : 128 * rt + 128],
                                    xc[:, c, :],
                                    start=(c == 0),
                                    stop=(c == 15),
                                )
                            dst = out_sb[:, rt, 512 * tt : 512 * tt + 512]
                            nc.scalar.activation(
                                dst, pp, mybir.ActivationFunctionType.Copy
                            )
                            drain(pool=pST, tag="pST")
                    # RoPE for this 512-token quarter, both heads at once:
                    # issued before the V chains so the DVE work hides under
                    # them and the last quarter's rope never delays attention
                    sl = slice(512 * tt, 512 * tt + 512)
                    for out_sb in (qt_sb, kt_sb):
                        rope_rows = out_sb[0:ROPE, :, sl]
                        swap = ropepool.tile([ROPE, HPC, 512], BF16, tag="rope")
                        nc.sync.dma_start(
                            out=swap[0:32], in_=out_sb[32:64, :, sl]
                        )
                        nc.sync.dma_start(
                            out=swap[32:64], in_=out_sb[0:32, :, sl]
                        )
                        nc.vector.tensor_mul(swap, swap, ss_sb[:, :, sl])
                        nc.vector.tensor_mul(
                            rope_rows, rope_rows, cc_sb[:, :, sl]
                        )
                        nc.vector.tensor_add(rope_rows, rope_rows, swap)

                    # V rows (natural layout), 4 row-groups of 128 tokens.
                    # The last group of the last quarter is deferred into the
                    # attention prologue so its PE work covers the first
                    # instance's exp latency.
                    def v_chain(xc_, tt_, g):
                        pv = pA.tile([128, 512], F32, tag="pA")
                        pvj = pv[:, 0:DQ]
                        for c in range(16):
                            nc.tensor.matmul(
                                pvj,
                                xc_[:, c, 128 * g : 128 * g + 128],
                                wv_sb[:, c, :],
                                start=(c == 0),
                                stop=(c == 15),
                            )
                        nc.vector.tensor_copy(v_sb[:, 4 * tt_ + g, :], pvj)
                        drain(pool=pST, tag="pST")

                    last_tt = tt == NTT - 1
                    for g in range(3 if last_tt else 4):
                        v_chain(xc, tt, g)
                    if last_tt:
                        deferred_v.append(
                            (lambda xc_=xc, tt_=tt: v_chain(xc_, tt_, 3))
                        )


                # ---------- attention (y interleaved via pending queue) --
                # One software pipeline ACROSS all (t, h) instances: produces
                # (ST+exp+mask) run DEPTH slots ahead of consumes (r/AV), so
                # the PE never drains at instance boundaries.
                DEPTH = 3
                av_rp = {}

                def produce(t, h, j):
                    q = j - 4 * t
                    lo = 128 * q if q > 0 else 0
                    st = pST.tile([128, 512], F32, tag="pST")
                    nc.tensor.matmul(
                        st[:, lo:512],
                        kt_sb[:, h, 128 * j : 128 * j + 128],
                        qt_sb[:, h, 512 * t + lo : 512 * t + 512],
                        start=True,
                        stop=True,
                    )
                    et = etpool.tile([128, 512], BF16, tag="et")
                    nc.scalar.activation(
                        et[:, lo:512],
                        st[:, lo:512],
                        mybir.ActivationFunctionType.Exp,
                    )
                    if q >= 0:
                        nc.vector.tensor_mul(
                            et[:, lo : lo + 128],
                            et[:, lo : lo + 128],
                            m1_sb,
                        )
                    return t, h, j, lo, et

                def consume(slot):
                    t, h, j, lo, et = slot
                    njb = 4 * t + 4
                    if j == 0:
                        av_rp[(t, h)] = (
                            pAV.tile([128, 512], F32, tag="pAV",
                                     name=f"av_{b}_{t}_{h}"),
                            pR.tile([128, 512], F32, tag="pR",
                                    name=f"rp_{b}_{t}_{h}"),
                        )
                    av, rp = av_rp[(t, h)]
                    # drain first: fills the PE while exp(j) finishes.
                    # t3 has 32 slots for 16 pendings: drain odd slots only
                    # so fill work lasts the whole instance
                    do = t < NI - 1 or j % 2 == 1
                    if b == B - 1 and t == NI - 1:
                        do = do and len(pending) > 4
                    if do:
                        drain()
                    nc.tensor.matmul(
                        rp[:, lo:512],
                        ones,
                        et[:, lo:512],
                        start=(j == 0),
                        stop=(j == njb - 1),
                    )
                    nc.tensor.matmul(
                        av[:, lo:512],
                        v_sb[:, j, HD * h : HD * h + HD],
                        et[:, lo:512],
                        start=(j == 0),
                        stop=(j == njb - 1),
                    )
                    if j == njb - 1:
                        av, rp = av_rp.pop((t, h))
                        rec = recpool.tile([128, 512], F32, tag="rec")
                        nc.vector.reciprocal(rec, rp)
                        nc.vector.tensor_tensor(
                            att_sb[:, h, 512 * t : 512 * t + 512],
                            av,
                            rec,
                            op=mybir.AluOpType.mult,
                        )
                        if h == HPC - 1:
                            pending.extend(
                                (b, t2, dd)
                                for t2 in range(4 * t, 4 * t + 4)
                                for dd in range(4)
                            )

                stream = [
                    (t, h, j)
                    for t in range(NI)
                    for h in range(HPC)
                    for j in range(4 * t + 4)
                ]
                window = []
                for n_, thj in enumerate(stream):
                    window.append(produce(*thj))
                    if n_ == DEPTH - 1 and deferred_v:
                        deferred_v.pop()()
                    if len(window) > DEPTH:
                        consume(window.pop(0))
                for slot in window:
                    consume(slot)
            # final drain (tail of last batch): ST/AV PSUM banks are free
            # here, so rotate across all pools to keep emissions in flight
            rot = [
                (pA, "pA"), (pST, "pST"), (pA, "pA"),
                (pST, "pST"), (pA, "pA"), (pAV, "pAV"),
            ]
            i = 0
            while pending:
                pool, tag = rot[i % len(rot)]
                emit_y(
                    *pending.pop(0), pool=pool, tag=tag,
                    split_dma=len(pending) < 4,
                )
                i += 1
    nc.compile()
    return nc


_NC = None


def _get_nc():
    global _NC
    if _NC is None:
        _NC = build_nc()
    return _NC


def _host_inputs(x, mask, wq, wk, wv, wo):
    import ml_dtypes

    x = np.asarray(x, np.float32)
    wq = np.asarray(wq, np.float32)
    wk = np.asarray(wk, np.float32)
    wv = np.asarray(wv, np.float32)
    wo = np.asarray(wo, np.float32)

    xt = np.ascontiguousarray(x.transpose(0, 2, 1)).astype(ml_dtypes.bfloat16)

    # permute head dims so RoPE pairs are (i, i+32): [evens, odds, pass-through]
    perm128 = np.concatenate(
        [np.arange(0, ROPE, 2), np.arange(1, ROPE, 2), np.arange(ROPE, HD)]
    )
    permD = np.concatenate([h * HD + perm128 for h in range(H)])
    wq_p = (wq * np.float32(1.0 / np.sqrt(HD)))[:, permD]
    wk_p = wk[:, permD]

    # RoPE tables, matching reference fp32 math (dim=64, repeat-2 interleave)
    # cc = [cos; cos], ss = [-sin; +sin] for the (x1=rows 0:32, x2=rows 32:64)
    # pairing: rot = [x1;x2]*cc + [x2;x1]*ss  (duplicated across head axis)
    ts_ = np.arange(0, ROPE, 2, dtype=np.float32)
    inv = (np.float32(10000.0) ** (-ts_ / np.float32(ROPE))).astype(np.float32)
    grid = np.arange(L, dtype=np.float32)[:, None] * inv[None, :]  # [L, 32]
    cc1 = np.empty((ROPE, L), np.float32)
    cc1[0:32] = cc1[32:64] = np.cos(grid).T
    ss1 = np.empty((ROPE, L), np.float32)
    ss1[0:32] = -np.sin(grid).T
    ss1[32:64] = np.sin(grid).T
    cc = np.repeat(cc1[:, None, :], HPC, axis=1).astype(ml_dtypes.bfloat16)
    ss = np.repeat(ss1[:, None, :], HPC, axis=1).astype(ml_dtypes.bfloat16)

    # strip mask for diagonal j-blocks: valid iff (i - 128q) >= j
    jj = np.arange(128)
    m1 = (jj[None, :] >= jj[:, None]).astype(ml_dtypes.bfloat16)

    in_maps = []
    for c in range(NCORES):
        sl = slice(DQ * c, DQ * c + DQ)
        in_maps.append(
            {
                "xt": xt,
                "wq": np.ascontiguousarray(wq_p[:, sl]).astype(
                    ml_dtypes.bfloat16
                ),
                "wk": np.ascontiguousarray(wk_p[:, sl]).astype(
                    ml_dtypes.bfloat16
                ),
                "wv": np.ascontiguousarray(wv[:, sl]).astype(
                    ml_dtypes.bfloat16
                ),
                "wo": np.ascontiguousarray(wo[sl, :]).astype(
                    ml_dtypes.bfloat16
                ),
                "cc": cc,
                "ss": ss,
                "m1": m1,
            }
        )
    return in_maps


def _reference_host(x, mask, wq, wk, wv, wo):
    """Exact-math numpy fallback (used only if the mask is not causal-tril)."""
    Hh, P = H, 64
    xx = np.asarray(x, np.float32)
    Bb, Ll, Dd = xx.shape
    K = Dd // Hh

    def rope(t):  # [b,h,s,d]
        d, s = t.shape[-1], t.shape[-2]
        ts_ = np.arange(0, d, 2, dtype=np.float32)
        inv = np.float32(10000.0) ** (-ts_ / np.float32(d))
        grid = np.arange(s, dtype=np.float32)[:, None] * inv[None, :]
        sin = np.repeat(np.sin(grid), 2, axis=-1)[None, None]
        cos = np.repeat(np.cos(grid), 2, axis=-1)[None, None]
        x1, x2 = t[..., ::2], t[..., 1::2]
        xs = np.stack([-x2, x1], axis=-1).reshape(t.shape)
        return t * cos + xs * sin

    def split(t):
        return t.reshape(Bb, Ll, Hh, K).transpose(0, 2, 1, 3)

    q = split(xx @ np.asarray(wq, np.float32)) / np.sqrt(K)
    q = np.concatenate([rope(q[..., :P]), q[..., P:]], axis=-1)
    k = split(xx @ np.asarray(wk, np.float32))
    k = np.concatenate([rope(k[..., :P]), k[..., P:]], axis=-1)
    v = split(xx @ np.asarray(wv, np.float32))
    s = np.einsum("bhik,bhjk->bhij", q, k)
    s = np.where(np.asarray(mask), s, np.float32(-1e8))
    s -= s.max(axis=-1, keepdims=True)
    e = np.exp(s)
    a = e / e.sum(axis=-1, keepdims=True)
    yy = np.einsum("bhij,bhjv->bhiv", a, v)
    yy = yy.transpose(0, 2, 1, 3).reshape(Bb, Ll, Dd)
    return (yy @ np.asarray(wo, np.float32)).astype(np.float32)


def kernel(**inputs):
    mask_arr = np.asarray(inputs["mask"])
    if not bool((mask_arr[0, 0] == np.tril(np.ones((L, L), bool))).all()):
        return _reference_host(
            inputs["x"], inputs["mask"], inputs["wq"], inputs["wk"],
            inputs["wv"], inputs["wo"],
        )
    nc = _get_nc()
    in_maps = _host_inputs(
        inputs["x"], inputs["mask"], inputs["wq"], inputs["wk"],
        inputs["wv"], inputs["wo"],
    )
    res = run_bass_kernel_spmd(nc, in_maps, core_ids=list(range(NCORES)))
    out = np.zeros((B, L, D), np.float64)
    for c in range(NCORES):
        out += np.asarray(res.results[c]["y"], np.float64)
    return out.astype(np.float32)


# revision 3
# speedup vs baseline: 1.0002x; 1.0002x over previous
"""Multi-head attention (B=2, L=2048, D=2048, H=16, causal + RoPE) on 8 TRN2 cores.

Sharding: tensor-parallel over heads. Core c owns heads {2c, 2c+1}:
  - wq/wk/wv column slices [D, 256], wo row slice [256, D]
  - each core computes a partial output y_c = att_c @ wo_c (full shape, bf16)
  - host reduces: y = sum_c y_c  (the "all-reduce" of the output projection)

v2 design (all-bf16 operands, fp32 PSUM accumulation):
  1. Projections: QT/KT = (w^T x^T) in transposed layout [head_dim, tok],
     V in natural layout [tok, head_dim]; all matmul operands bf16 (1 cyc/row).
     RoPE applied to QT/KT rows 0:64 per head with head-duplicated cc/ss
     tables so each DVE op covers both heads ([64, 2, 512], bf16 2x mode).
  2. Attention per (batch, head): causal, no-max-subtraction softmax.
     Per j-block of 128 keys against a 512-query i-tile:
       ST = K_j^T Q_i   (PE)     -- diagonal blocks narrowed to valid columns
       ET = exp(ST)     (ACT, bf16 out) ; diagonal strip masked via one
                                  [128,128] tril mult (DVE, bf16 2x)
       R  += ones^T ET  (PE)     -- ones stationary is [128,128] so R PSUM
                                    holds denominators broadcast to all
                                    partitions: no transpose bounce needed
       AV += V_j^T ET   (PE)
     Then att[:, h, i-tile] = AV * reciprocal(R) -- two DVE ops, fused
     normalize + PSUM->SBUF copy.
  3. Output: y[tok,:] = sum_h att_h^T wo_h ; both heads accumulate into one
     PSUM bank; PSUM->SBUF copies alternate ACT/DVE into a [128, 4, 512]
     bf16 stage written out as one [128, 2048] DMA per 128-token chunk.
     Emissions are interleaved ("pending" queue) into later attention
     j-blocks and the next batch's projection chains so the PE never idles
     through an emission-only phase.
"""

import glob
import os


def _ensure_env():
    # walrus_driver (neuronx-cc) must be on PATH for client-side NEFF compile.
    if not any("-b16-bazel-" in p for p in os.environ.get("PATH", "").split(":")):
        cands = sorted(glob.glob("/nix/store/*-b16-bazel-*/bin"))
        for c in cands:
            if os.path.exists(os.path.join(c, "neuronx-cc")) or glob.glob(
                os.path.join(c, "*walrus*")
            ):
                os.environ["PATH"] = c + ":" + os.environ["PATH"]
                break
        else:
            if cands:
                os.environ["PATH"] = cands[-1] + ":" + os.environ["PATH"]


_ensure_env()
os.environ.setdefault("JAX_COMPILATION_CACHE_DIR", "/tmp/jax_comp_cache")
os.environ.setdefault("JAX_PERSISTENT_CACHE_MIN_COMPILE_TIME_SECS", "1")
os.environ.setdefault("JAX_PERSISTENT_CACHE_MIN_ENTRY_SIZE_BYTES", "0")

import numpy as np  # noqa: E402

import concourse.bass as bass  # noqa: E402
import concourse.mybir as mybir  # noqa: E402
import concourse.tile as tile  # noqa: E402
from concourse import bacc  # noqa: E402
from concourse.bass_utils import run_bass_kernel_spmd  # noqa: E402

NCORES = 8
B, L, D = 2, 2048, 2048
H = 16
HD = 128            # head dim
HPC = H // NCORES   # heads per core
DQ = HPC * HD       # 256: per-core projection width
ROPE = 64           # RoPE dims per head
F32 = mybir.dt.float32
BF16 = mybir.dt.bfloat16

NTT = L // 512      # 4 token tiles (512) for projections
NI = L // 512       # 4 i-tiles per attention instance
NJ = L // 128       # 16 j-blocks


def build_nc():
    nc = bacc.Bacc(
        "TRN2", target_bir_lowering=False, debug=False, num_devices=NCORES
    )
    xt = nc.dram_tensor("xt", [B, D, L], BF16, kind="ExternalInput").ap()
    wq = nc.dram_tensor("wq", [D, DQ], BF16, kind="ExternalInput").ap()
    wk = nc.dram_tensor("wk", [D, DQ], BF16, kind="ExternalInput").ap()
    wv = nc.dram_tensor("wv", [D, DQ], BF16, kind="ExternalInput").ap()
    wo = nc.dram_tensor("wo", [DQ, D], BF16, kind="ExternalInput").ap()
    # cc rows = [cos;cos], ss rows = [-sin;+sin], duplicated per head so one
    # DVE op can cover both heads' rope rows: [ROPE, HPC, L]
    cc = nc.dram_tensor("cc", [ROPE, HPC, L], BF16, kind="ExternalInput").ap()
    ss = nc.dram_tensor("ss", [ROPE, HPC, L], BF16, kind="ExternalInput").ap()
    # strip mask for diagonal j-blocks: m1[j, x] = 1.0 if x >= j else 0
    m1 = nc.dram_tensor("m1", [128, 128], BF16, kind="ExternalInput").ap()
    y = nc.dram_tensor("y", [B, L, D], BF16, kind="ExternalOutput").ap()

    with tile.TileContext(nc) as tc:
        with (
            tc.tile_pool(name="consts", bufs=1) as consts,
            tc.tile_pool(name="wpool", bufs=1) as wpool,
            tc.tile_pool(name="qkv", bufs=1) as qkv,
            tc.tile_pool(name="xc", bufs=2) as xcpool,
            tc.tile_pool(name="et", bufs=6) as etpool,
            tc.tile_pool(name="rope", bufs=4) as ropepool,
            tc.tile_pool(name="rec", bufs=2) as recpool,
            tc.tile_pool(name="ysb", bufs=4) as ypool,
            tc.tile_pool(name="pA", bufs=2, space="PSUM") as pA,
            tc.tile_pool(name="pST", bufs=3, space="PSUM") as pST,
            tc.tile_pool(name="pAV", bufs=2, space="PSUM") as pAV,
            tc.tile_pool(name="pR", bufs=1, space="PSUM") as pR,
        ):
            # ---- constants / weights ----
            ones = consts.tile([128, 128], BF16)
            nc.vector.memset(ones, 1.0)
            # warm the PE p-state during the startup DMA wait: tiny matmuls
            # keep the tensor engine continuously busy so the first real
            # chains run at full clock (ramp needs 3us of busy)
            pwarm = pR.tile([128, 64], F32, tag="pR", name="pwarm")
            for _ in range(60):
                nc.tensor.matmul(
                    pwarm[0:1, :], ones[:, 0:1], ones[:, 0:64],
                    start=True, stop=True,
                )
            m1_sb = consts.tile([128, 128], BF16)
            cc_sb = consts.tile([ROPE, HPC, L], BF16)
            ss_sb = consts.tile([ROPE, HPC, L], BF16)
            xc_pre = xcpool.tile([128, 16, 512], BF16, tag="xc")
            wq_sb = wpool.tile([128, 16, DQ], BF16)
            wk_sb = wpool.tile([128, 16, DQ], BF16)
            wv_sb = wpool.tile([128, 16, DQ], BF16)
            wo_sb = wpool.tile([128, HPC, D], BF16)

            def _w_oct(w_dram, w_sb, oct_):
                nc.sync.dma_start(
                    out=w_sb[:, 2 * oct_ : 2 * oct_ + 2, :],
                    in_=w_dram[256 * oct_ : 256 * oct_ + 256, :].rearrange(
                        "(c p) o -> p c o", p=128
                    ),
                )

            # DMA order = first-use order: the first q-projection chain needs
            # xc oct_c + wq oct_c in sequence, so interleave them; wk/wv feed
            # the following chains; rope tables before the first rope; wo and
            # the mask strip are only needed tens of us in.
            for oct_ in range(8):
                nc.sync.dma_start(
                    out=xc_pre[:, 2 * oct_ : 2 * oct_ + 2, :],
                    in_=xt[0, 256 * oct_ : 256 * oct_ + 256, 0:512].rearrange(
                        "(c p) t -> p c t", p=128
                    ),
                )
                _w_oct(wq, wq_sb, oct_)
            for w_dram, w_sb in ((wk, wk_sb), (wv, wv_sb)):
                for qq in range(4):
                    nc.sync.dma_start(
                        out=w_sb[:, 4 * qq : 4 * qq + 4, :],
                        in_=w_dram[
                            512 * qq : 512 * qq + 512, :
                        ].rearrange("(c p) o -> p c o", p=128),
                    )

            qt_sb = qkv.tile([128, HPC, L], BF16)   # [d, h, tok]
            kt_sb = qkv.tile([128, HPC, L], BF16)
            v_sb = qkv.tile([128, NJ, DQ], BF16)    # [tok_in_blk, jblk, hd]
            att_sb = qkv.tile([128, HPC, L], BF16)  # [hd, h, tok] normalized

            # ---- y emission machinery ----
            pending = []          # (b, t2) chunks with dd sub-emissions
            ystage = {}           # t2 -> stage tile
            ecount = [0]          # emission counter for ACT/DVE alternation

            def emit_y(b_, t2, dd, pool=None, tag="pA", eng="alt", split_dma=False):
                pool = pool or pA
                p = pool.tile([128, 512], F32, tag=tag, name=f"yp_{b_}_{t2}_{dd}")
                nc.tensor.matmul(
                    p,
                    att_sb[:, 0, 128 * t2 : 128 * t2 + 128],
                    wo_sb[:, 0, 512 * dd : 512 * dd + 512],
                    start=True,
                    stop=False,
                )
                nc.tensor.matmul(
                    p,
                    att_sb[:, 1, 128 * t2 : 128 * t2 + 128],
                    wo_sb[:, 1, 512 * dd : 512 * dd + 512],
                    start=False,
                    stop=True,
                )
                if dd == 0:
                    ystage[t2] = ypool.tile(
                        [128, 4, 512], BF16, tag="ysb", name=f"yst_{b_}_{t2}"
                    )
                dst = ystage[t2][:, dd, :]
                # ACT is the exp critical path during attention: route those
                # drains' copies to DVE; alternate engines elsewhere
                if eng == "dve" or (eng == "alt" and ecount[0] % 2 == 1):
                    nc.vector.tensor_copy(dst, p)
                else:
                    nc.scalar.activation(
                        dst, p, mybir.ActivationFunctionType.Copy
                    )
                ecount[0] += 1
                if split_dma:
                    qeng = (nc.gpsimd, nc.scalar, nc.gpsimd, nc.sync)[dd]
                    qeng.dma_start(
                        out=y[
                            b_,
                            128 * t2 : 128 * t2 + 128,
                            512 * dd : 512 * dd + 512,
                        ],
                        in_=dst,
                    )
                    if dd == 3:
                        ystage.pop(t2)
                elif dd == 3:
                    nc.gpsimd.dma_start(
                        out=y[b_, 128 * t2 : 128 * t2 + 128, :],
                        in_=ystage.pop(t2),
                    )

            def drain(n=1, eng="alt", pool=None, tag="pA"):
                for _ in range(n):
                    if pending:
                        emit_y(*pending.pop(0), eng=eng, pool=pool, tag=tag)

            xc_tiles = {(0, 0): xc_pre}

            def xc_load(b_, tt_):
                # prefetch: issue the x-chunk DMAs one tile ahead of use
                if (b_, tt_) in xc_tiles or b_ >= B:
                    return
                xct = xcpool.tile(
                    [128, 16, 512], BF16, tag="xc", name=f"xc_{b_}_{tt_}"
                )
                for qq in range(4):
                    nc.sync.dma_start(
                        out=xct[:, 4 * qq : 4 * qq + 4, :],
                        in_=xt[
                            b_,
                            512 * qq : 512 * qq + 512,
                            512 * tt_ : 512 * tt_ + 512,
                        ].rearrange("(c p) t -> p c t", p=128),
                    )
                xc_tiles[(b_, tt_)] = xct

            deferred_v = []

            for b in range(B):
                # ---------- projections ----------
                for tt in range(NTT):
                    xc_load(b, tt)
                    xc = xc_tiles.pop((b, tt))
                    if tt + 1 < NTT:
                        xc_load(b, tt + 1)
                    else:
                        xc_load(b + 1, 0)
                    if b == 0 and tt == 0:
                        # tables/masks/wo are needed late; issue after the
                        # tt1 x prefetch so that lands first
                        nc.sync.dma_start(out=cc_sb, in_=cc)
                        nc.sync.dma_start(out=ss_sb, in_=ss)
                        nc.sync.dma_start(out=m1_sb, in_=m1)
                        for hh in range(HPC):
                            nc.sync.dma_start(
                                out=wo_sb[:, hh, :],
                                in_=wo[HD * hh : HD * hh + HD, :],
                            )
                    # QT / KT rows (transposed layout); RoPE applied below
                    for w_sb, out_sb in (
                        (wq_sb, qt_sb),
                        (wk_sb, kt_sb),
                    ):
                        for rt in range(HPC):
                            pp = pA.tile([128, 512], F32, tag="pA")
                            for c in range(16):
                                nc.tensor.matmul(
                                    pp,
                                    w_sb[:, c, 128 * rt : 128 * rt + 128],
                                    xc[:, c, :],
                                    start=(c == 0),
                                    stop=(c == 15),
                                )
                            dst = out_sb[:, rt, 512 * tt : 512 * tt + 512]
                            nc.scalar.activation(
                                dst, pp, mybir.ActivationFunctionType.Copy
                            )
                            drain(pool=pST, tag="pST")
                    # RoPE for this 512-token quarter, both heads at once:
                    # issued before the V chains so the DVE work hides under
                    # them and the last quarter's rope never delays attention
                    sl = slice(512 * tt, 512 * tt + 512)
                    for out_sb in (qt_sb, kt_sb):
                        rope_rows = out_sb[0:ROPE, :, sl]
                        swap = ropepool.tile([ROPE, HPC, 512], BF16, tag="rope")
                        nc.sync.dma_start(
                            out=swap[0:32], in_=out_sb[32:64, :, sl]
                        )
                        nc.sync.dma_start(
                            out=swap[32:64], in_=out_sb[0:32, :, sl]
                        )
                        nc.vector.tensor_mul(swap, swap, ss_sb[:, :, sl])
                        nc.vector.tensor_mul(
                            rope_rows, rope_rows, cc_sb[:, :, sl]
                        )
                        nc.vector.tensor_add(rope_rows, rope_rows, swap)

                    # V rows (natural layout), 4 row-groups of 128 tokens.
                    # The last group of the last quarter is deferred into the
                    # attention prologue so its PE work covers the first
                    # instance's exp latency.
                    def v_chain(xc_, tt_, g):
                        pv = pA.tile([128, 512], F32, tag="pA")
                        pvj = pv[:, 0:DQ]
                        for c in range(16):
                            nc.tensor.matmul(
                                pvj,
                                xc_[:, c, 128 * g : 128 * g + 128],
                                wv_sb[:, c, :],
                                start=(c == 0),
                                stop=(c == 15),
                            )
                        nc.vector.tensor_copy(v_sb[:, 4 * tt_ + g, :], pvj)
                        drain(pool=pST, tag="pST")

                    last_tt = tt == NTT - 1
                    for g in range(3 if last_tt else 4):
                        v_chain(xc, tt, g)
                    if last_tt:
                        deferred_v.append(
                            (lambda xc_=xc, tt_=tt: v_chain(xc_, tt_, 3))
                        )


                # ---------- attention (y interleaved via pending queue) --
                # One software pipeline ACROSS all (t, h) instances: produces
                # (ST+exp+mask) run DEPTH slots ahead of consumes (r/AV), so
                # the PE never drains at instance boundaries.
                DEPTH = 3
                av_rp = {}

                def produce(t, h, j):
                    q = j - 4 * t
                    lo = 128 * q if q > 0 else 0
                    st = pST.tile([128, 512], F32, tag="pST")
                    nc.tensor.matmul(
                        st[:, lo:512],
                        kt_sb[:, h, 128 * j : 128 * j + 128],
                        qt_sb[:, h, 512 * t + lo : 512 * t + 512],
                        start=True,
                        stop=True,
                    )
                    et = etpool.tile([128, 512], BF16, tag="et")
                    nc.scalar.activation(
                        et[:, lo:512],
                        st[:, lo:512],
                        mybir.ActivationFunctionType.Exp,
                    )
                    if q >= 0:
                        nc.vector.tensor_mul(
                            et[:, lo : lo + 128],
                            et[:, lo : lo + 128],
                            m1_sb,
                        )
                    return t, h, j, lo, et

                def consume(slot):
                    t, h, j, lo, et = slot
                    njb = 4 * t + 4
                    if j == 0:
                        av_rp[(t, h)] = (
                            pAV.tile([128, 512], F32, tag="pAV",
                                     name=f"av_{b}_{t}_{h}"),
                            pR.tile([128, 512], F32, tag="pR",
                                    name=f"rp_{b}_{t}_{h}"),
                        )
                    av, rp = av_rp[(t, h)]
                    # drain first: fills the PE while exp(j) finishes.
                    # t3 has 32 slots for 16 pendings: drain odd slots only
                    # so fill work lasts the whole instance
                    do = t < NI - 1 or j % 2 == 1
                    if b == B - 1 and t == NI - 1:
                        do = do and len(pending) > 4
                    if do:
                        drain()
                    nc.tensor.matmul(
                        rp[:, lo:512],
                        ones,
                        et[:, lo:512],
                        start=(j == 0),
                        stop=(j == njb - 1),
                    )
                    nc.tensor.matmul(
                        av[:, lo:512],
                        v_sb[:, j, HD * h : HD * h + HD],
                        et[:, lo:512],
                        start=(j == 0),
                        stop=(j == njb - 1),
                    )
                    if j == njb - 1:
                        av, rp = av_rp.pop((t, h))
                        rec = recpool.tile([128, 512], F32, tag="rec")
                        nc.vector.reciprocal(rec, rp)
                        nc.vector.tensor_tensor(
                            att_sb[:, h, 512 * t : 512 * t + 512],
                            av,
                            rec,
                            op=mybir.AluOpType.mult,
                        )
                        if h == HPC - 1:
                            pending.extend(
                                (b, t2, dd)
                                for t2 in range(4 * t, 4 * t + 4)
                                for dd in range(4)
                            )

                stream = [
                    (t, h, j)
                    for t in range(NI)
                    for h in range(HPC)
                    for j in range(4 * t + 4)
                ]
                window = []
                for n_, thj in enumerate(stream):
                    window.append(produce(*thj))
                    if n_ == DEPTH - 1 and deferred_v:
                        deferred_v.pop()()
                    if len(window) > DEPTH:
                        consume(window.pop(0))
                for slot in window:
                    consume(slot)
            # final drain (tail of last batch): ST/AV PSUM banks are free
            # here, so rotate across all pools to keep emissions in flight
            rot = [
                (pA, "pA"), (pST, "pST"), (pA, "pA"),
                (pST, "pST"), (pA, "pA"), (pAV, "pAV"),
            ]
            i = 0
            while pending:
                pool, tag = rot[i % len(rot)]
                emit_y(
                    *pending.pop(0), pool=pool, tag=tag,
                    split_dma=len(pending) < 4,
                )
                i += 1
    nc.compile()
    return nc


_NC = None


def _get_nc():
    global _NC
    if _NC is None:
        _NC = build_nc()
    return _NC


def _host_inputs(x, mask, wq, wk, wv, wo):
    import ml_dtypes

    x = np.asarray(x, np.float32)
    wq = np.asarray(wq, np.float32)
    wk = np.asarray(wk, np.float32)
    wv = np.asarray(wv, np.float32)
    wo = np.asarray(wo, np.float32)

    xt = np.ascontiguousarray(x.transpose(0, 2, 1)).astype(ml_dtypes.bfloat16)

    # permute head dims so RoPE pairs are (i, i+32): [evens, odds, pass-through]
    perm128 = np.concatenate(
        [np.arange(0, ROPE, 2), np.arange(1, ROPE, 2), np.arange(ROPE, HD)]
    )
    permD = np.concatenate([h * HD + perm128 for h in range(H)])
    wq_p = (wq * np.float32(1.0 / np.sqrt(HD)))[:, permD]
    wk_p = wk[:, permD]

    # RoPE tables, matching reference fp32 math (dim=64, repeat-2 interleave)
    # cc = [cos; cos], ss = [-sin; +sin] for the (x1=rows 0:32, x2=rows 32:64)
    # pairing: rot = [x1;x2]*cc + [x2;x1]*ss  (duplicated across head axis)
    ts_ = np.arange(0, ROPE, 2, dtype=np.float32)
    inv = (np.float32(10000.0) ** (-ts_ / np.float32(ROPE))).astype(np.float32)
    grid = np.arange(L, dtype=np.float32)[:, None] * inv[None, :]  # [L, 32]
    cc1 = np.empty((ROPE, L), np.float32)
    cc1[0:32] = cc1[32:64] = np.cos(grid).T
    ss1 = np.empty((ROPE, L), np.float32)
    ss1[0:32] = -np.sin(grid).T
    ss1[32:64] = np.sin(grid).T
    cc = np.repeat(cc1[:, None, :], HPC, axis=1).astype(ml_dtypes.bfloat16)
    ss = np.repeat(ss1[:, None, :], HPC, axis=1).astype(ml_dtypes.bfloat16)

    # strip mask for diagonal j-blocks: valid iff (i - 128q) >= j
    jj = np.arange(128)
    m1 = (jj[None, :] >= jj[:, None]).astype(ml_dtypes.bfloat16)

    in_maps = []
    for c in range(NCORES):
        sl = slice(DQ * c, DQ * c + DQ)
        in_maps.append(
            {
                "xt": xt,
                "wq": np.ascontiguousarray(wq_p[:, sl]).astype(
                    ml_dtypes.bfloat16
                ),
                "wk": np.ascontiguousarray(wk_p[:, sl]).astype(
                    ml_dtypes.bfloat16
                ),
                "wv": np.ascontiguousarray(wv[:, sl]).astype(
                    ml_dtypes.bfloat16
                ),
                "wo": np.ascontiguousarray(wo[sl, :]).astype(
                    ml_dtypes.bfloat16
                ),
                "cc": cc,
                "ss": ss,
                "m1": m1,
            }
        )
    return in_maps


def _reference_host(x, mask, wq, wk, wv, wo):
    """Exact-math numpy fallback (used only if the mask is not causal-tril)."""
    Hh, P = H, 64
    xx = np.asarray(x, np.float32)
    Bb, Ll, Dd = xx.shape
    K = Dd // Hh

    def rope(t):  # [b,h,s,d]
        d, s = t.shape[-1], t.shape[-2]
        ts_ = np.arange(0, d, 2, dtype=np.float32)
        inv = np.float32(10000.0) ** (-ts_ / np.float32(d))
        grid = np.arange(s, dtype=np.float32)[:, None] * inv[None, :]
        sin = np.repeat(np.sin(grid), 2, axis=-1)[None, None]
        cos = np.repeat(np.cos(grid), 2, axis=-1)[None, None]
        x1, x2 = t[..., ::2], t[..., 1::2]
        xs = np.stack([-x2, x1], axis=-1).reshape(t.shape)
        return t * cos + xs * sin

    def split(t):
        return t.reshape(Bb, Ll, Hh, K).transpose(0, 2, 1, 3)

    q = split(xx @ np.asarray(wq, np.float32)) / np.sqrt(K)
    q = np.concatenate([rope(q[..., :P]), q[..., P:]], axis=-1)
    k = split(xx @ np.asarray(wk, np.float32))
    k = np.concatenate([rope(k[..., :P]), k[..., P:]], axis=-1)
    v = split(xx @ np.asarray(wv, np.float32))
    s = np.einsum("bhik,bhjk->bhij", q, k)
    s = np.where(np.asarray(mask), s, np.float32(-1e8))
    s -= s.max(axis=-1, keepdims=True)
    e = np.exp(s)
    a = e / e.sum(axis=-1, keepdims=True)
    yy = np.einsum("bhij,bhjv->bhiv", a, v)
    yy = yy.transpose(0, 2, 1, 3).reshape(Bb, Ll, Dd)
    return (yy @ np.asarray(wo, np.float32)).astype(np.float32)


def kernel(**inputs):
    mask_arr = np.asarray(inputs["mask"])
    if not bool((mask_arr[0, 0] == np.tril(np.ones((L, L), bool))).all()):
        return _reference_host(
            inputs["x"], inputs["mask"], inputs["wq"], inputs["wk"],
            inputs["wv"], inputs["wo"],
        )
    nc = _get_nc()
    in_maps = _host_inputs(
        inputs["x"], inputs["mask"], inputs["wq"], inputs["wk"],
        inputs["wv"], inputs["wo"],
    )
    res = run_bass_kernel_spmd(nc, in_maps, core_ids=list(range(NCORES)))
    out = np.zeros((B, L, D), np.float64)
    for c in range(NCORES):
        out += np.asarray(res.results[c]["y"], np.float64)
    return out.astype(np.float32)


# revision 5
# speedup vs baseline: 1.0334x; 1.0332x over previous
"""Multi-head attention (B=2, L=2048, D=2048, H=16, causal + RoPE) on 8 TRN2 cores.

Sharding: tensor-parallel over heads. Core c owns heads {2c, 2c+1}:
  - wq/wk/wv column slices [D, 256], wo row slice [256, D]
  - each core computes a partial output y_c = att_c @ wo_c (full shape, bf16)
  - host reduces: y = sum_c y_c  (the "all-reduce" of the output projection)

v2 design (all-bf16 operands, fp32 PSUM accumulation):
  1. Projections: QT/KT = (w^T x^T) in transposed layout [head_dim, tok],
     V in natural layout [tok, head_dim]; all matmul operands bf16 (1 cyc/row).
     RoPE applied to QT/KT rows 0:64 per head with head-duplicated cc/ss
     tables so each DVE op covers both heads ([64, 2, 512], bf16 2x mode).
  2. Attention per (batch, head): causal, no-max-subtraction softmax.
     Per j-block of 128 keys against a 512-query i-tile:
       ST = K_j^T Q_i   (PE)     -- diagonal blocks narrowed to valid columns
       ET = exp(ST)     (ACT, bf16 out) ; diagonal strip masked via one
                                  [128,128] tril mult (DVE, bf16 2x)
       R  += ones^T ET  (PE)     -- ones stationary is [128,128] so R PSUM
                                    holds denominators broadcast to all
                                    partitions: no transpose bounce needed
       AV += V_j^T ET   (PE)
     Then att[:, h, i-tile] = AV * reciprocal(R) -- two DVE ops, fused
     normalize + PSUM->SBUF copy.
  3. Output: y[tok,:] = sum_h att_h^T wo_h ; both heads accumulate into one
     PSUM bank; PSUM->SBUF copies alternate ACT/DVE into a [128, 4, 512]
     bf16 stage written out as one [128, 2048] DMA per 128-token chunk.
     Emissions are interleaved ("pending" queue) into later attention
     j-blocks and the next batch's projection chains so the PE never idles
     through an emission-only phase.
"""

import glob
import os


def _ensure_env():
    # walrus_driver (neuronx-cc) must be on PATH for client-side NEFF compile.
    if not any("-b16-bazel-" in p for p in os.environ.get("PATH", "").split(":")):
        cands = sorted(glob.glob("/nix/store/*-b16-bazel-*/bin"))
        for c in cands:
            if os.path.exists(os.path.join(c, "neuronx-cc")) or glob.glob(
                os.path.join(c, "*walrus*")
            ):
                os.environ["PATH"] = c + ":" + os.environ["PATH"]
                break
        else:
            if cands:
                os.environ["PATH"] = cands[-1] + ":" + os.environ["PATH"]


_ensure_env()
os.environ.setdefault("JAX_COMPILATION_CACHE_DIR", "/tmp/jax_comp_cache")
os.environ.setdefault("JAX_PERSISTENT_CACHE_MIN_COMPILE_TIME_SECS", "1")
os.environ.setdefault("JAX_PERSISTENT_CACHE_MIN_ENTRY_SIZE_BYTES", "0")

import numpy as np  # noqa: E402

import concourse.bass as bass  # noqa: E402
import concourse.mybir as mybir  # noqa: E402
import concourse.tile as tile  # noqa: E402
from concourse import bacc  # noqa: E402
from concourse.bass_utils import run_bass_kernel_spmd  # noqa: E402

NCORES = 8
B, L, D = 2, 2048, 2048
H = 16
HD = 128            # head dim
HPC = H // NCORES   # heads per core
DQ = HPC * HD       # 256: per-core projection width
ROPE = 64           # RoPE dims per head
F32 = mybir.dt.float32
BF16 = mybir.dt.bfloat16

NTT = L // 512      # 4 token tiles (512) for projections
NI = L // 512       # 4 i-tiles per attention instance
NJ = L // 128       # 16 j-blocks


def build_nc():
    nc = bacc.Bacc(
        "TRN2", target_bir_lowering=False, debug=False, num_devices=NCORES
    )
    xt = nc.dram_tensor("xt", [B, D, L], BF16, kind="ExternalInput").ap()
    wq = nc.dram_tensor("wq", [D, DQ], BF16, kind="ExternalInput").ap()
    wk = nc.dram_tensor("wk", [D, DQ], BF16, kind="ExternalInput").ap()
    wv = nc.dram_tensor("wv", [D, DQ], BF16, kind="ExternalInput").ap()
    wo = nc.dram_tensor("wo", [DQ, D], BF16, kind="ExternalInput").ap()
    # cc rows = [cos;cos], ss rows = [-sin;+sin], duplicated per head so one
    # DVE op can cover both heads' rope rows: [ROPE, HPC, L]
    cc = nc.dram_tensor("cc", [ROPE, HPC, L], BF16, kind="ExternalInput").ap()
    ss = nc.dram_tensor("ss", [ROPE, HPC, L], BF16, kind="ExternalInput").ap()
    # strip mask for diagonal j-blocks: m1[j, x] = 1.0 if x >= j else 0
    m1 = nc.dram_tensor("m1", [128, 128], BF16, kind="ExternalInput").ap()
    y = nc.dram_tensor("y", [B, L, D], BF16, kind="ExternalOutput").ap()

    with tile.TileContext(nc) as tc:
        with (
            tc.tile_pool(name="consts", bufs=1) as consts,
            tc.tile_pool(name="wpool", bufs=1) as wpool,
            tc.tile_pool(name="qkv", bufs=1) as qkv,
            tc.tile_pool(name="xc", bufs=2) as xcpool,
            tc.tile_pool(name="et", bufs=6) as etpool,
            tc.tile_pool(name="es", bufs=3) as espool,
            tc.tile_pool(name="rope", bufs=4) as ropepool,
            tc.tile_pool(name="rec", bufs=2) as recpool,
            tc.tile_pool(name="ysb", bufs=4) as ypool,
            tc.tile_pool(name="pA", bufs=2, space="PSUM") as pA,
            tc.tile_pool(name="pST", bufs=3, space="PSUM") as pST,
            tc.tile_pool(name="pAV", bufs=2, space="PSUM") as pAV,
            tc.tile_pool(name="pR", bufs=1, space="PSUM") as pR,
        ):
            # ---- constants / weights ----
            ones = consts.tile([128, 128], BF16)
            nc.vector.memset(ones, 1.0)
            # warm the PE p-state during the startup DMA wait: tiny matmuls
            # keep the tensor engine continuously busy so the first real
            # chains run at full clock (ramp needs 3us of busy)
            pwarm = pR.tile([128, 64], F32, tag="pR", name="pwarm")
            for _ in range(60):
                nc.tensor.matmul(
                    pwarm[0:1, :], ones[:, 0:1], ones[:, 0:64],
                    start=True, stop=True,
                )
            m1_sb = consts.tile([128, 128], BF16)
            cc_sb = consts.tile([ROPE, HPC, L], BF16)
            ss_sb = consts.tile([ROPE, HPC, L], BF16)
            xc_pre = xcpool.tile([128, 16, 512], BF16, tag="xc")
            wq_sb = wpool.tile([128, 16, DQ], BF16)
            wk_sb = wpool.tile([128, 16, DQ], BF16)
            wv_sb = wpool.tile([128, 16, DQ], BF16)
            wo_sb = wpool.tile([128, HPC, D], BF16)

            def _w_oct(w_dram, w_sb, oct_):
                nc.sync.dma_start(
                    out=w_sb[:, 2 * oct_ : 2 * oct_ + 2, :],
                    in_=w_dram[256 * oct_ : 256 * oct_ + 256, :].rearrange(
                        "(c p) o -> p c o", p=128
                    ),
                )

            # DMA order = first-use order: the first q-projection chain needs
            # xc oct_c + wq oct_c in sequence, so interleave them; wk/wv feed
            # the following chains; rope tables before the first rope; wo and
            # the mask strip are only needed tens of us in.
            for oct_ in range(8):
                nc.sync.dma_start(
                    out=xc_pre[:, 2 * oct_ : 2 * oct_ + 2, :],
                    in_=xt[0, 256 * oct_ : 256 * oct_ + 256, 0:512].rearrange(
                        "(c p) t -> p c t", p=128
                    ),
                )
                _w_oct(wq, wq_sb, oct_)
            for w_dram, w_sb in ((wk, wk_sb), (wv, wv_sb)):
                for qq in range(4):
                    nc.sync.dma_start(
                        out=w_sb[:, 4 * qq : 4 * qq + 4, :],
                        in_=w_dram[
                            512 * qq : 512 * qq + 512, :
                        ].rearrange("(c p) o -> p c o", p=128),
                    )

            qt_sb = qkv.tile([128, HPC, L], BF16)   # [d, h, tok]
            kt_sb = qkv.tile([128, HPC, L], BF16)
            v_sb = qkv.tile([128, NJ, DQ], BF16)    # [tok_in_blk, jblk, hd]
            att_sb = qkv.tile([128, HPC, L], BF16)  # [hd, h, tok] normalized

            # ---- y emission machinery ----
            pending = []          # (b, t2) chunks with dd sub-emissions
            ystage = {}           # t2 -> stage tile
            ecount = [0]          # emission counter for ACT/DVE alternation

            def emit_y(b_, t2, dd, pool=None, tag="pA", eng="alt", split_dma=False):
                pool = pool or pA
                p = pool.tile([128, 512], F32, tag=tag, name=f"yp_{b_}_{t2}_{dd}")
                nc.tensor.matmul(
                    p,
                    att_sb[:, 0, 128 * t2 : 128 * t2 + 128],
                    wo_sb[:, 0, 512 * dd : 512 * dd + 512],
                    start=True,
                    stop=False,
                )
                nc.tensor.matmul(
                    p,
                    att_sb[:, 1, 128 * t2 : 128 * t2 + 128],
                    wo_sb[:, 1, 512 * dd : 512 * dd + 512],
                    start=False,
                    stop=True,
                )
                if dd == 0:
                    ystage[t2] = ypool.tile(
                        [128, 4, 512], BF16, tag="ysb", name=f"yst_{b_}_{t2}"
                    )
                dst = ystage[t2][:, dd, :]
                # ACT is the exp critical path during attention: route those
                # drains' copies to DVE; alternate engines elsewhere
                if eng == "dve" or (eng == "alt" and ecount[0] % 2 == 1):
                    nc.vector.tensor_copy(dst, p)
                else:
                    nc.scalar.activation(
                        dst, p, mybir.ActivationFunctionType.Copy
                    )
                ecount[0] += 1
                if split_dma:
                    qeng = (nc.gpsimd, nc.sync, nc.gpsimd, nc.sync)[dd]
                    qeng.dma_start(
                        out=y[
                            b_,
                            128 * t2 : 128 * t2 + 128,
                            512 * dd : 512 * dd + 512,
                        ],
                        in_=dst,
                    )
                    if dd == 3:
                        ystage.pop(t2)
                elif dd == 3:
                    nc.gpsimd.dma_start(
                        out=y[b_, 128 * t2 : 128 * t2 + 128, :],
                        in_=ystage.pop(t2),
                    )

            def drain(n=1, eng="alt", pool=None, tag="pA"):
                for _ in range(n):
                    if pending:
                        emit_y(*pending.pop(0), eng=eng, pool=pool, tag=tag)

            xc_tiles = {(0, 0): xc_pre}

            def xc_load(b_, tt_):
                # prefetch: issue the x-chunk DMAs one tile ahead of use
                if (b_, tt_) in xc_tiles or b_ >= B:
                    return
                xct = xcpool.tile(
                    [128, 16, 512], BF16, tag="xc", name=f"xc_{b_}_{tt_}"
                )
                for qq in range(4):
                    nc.sync.dma_start(
                        out=xct[:, 4 * qq : 4 * qq + 4, :],
                        in_=xt[
                            b_,
                            512 * qq : 512 * qq + 512,
                            512 * tt_ : 512 * tt_ + 512,
                        ].rearrange("(c p) t -> p c t", p=128),
                    )
                xc_tiles[(b_, tt_)] = xct

            deferred_v = []

            for b in range(B):
                # ---------- projections ----------
                for tt in range(NTT):
                    xc_load(b, tt)
                    xc = xc_tiles.pop((b, tt))
                    if tt + 1 < NTT:
                        xc_load(b, tt + 1)
                    else:
                        xc_load(b + 1, 0)
                    if b == 0 and tt == 0:
                        # tables/masks/wo are needed late; issue after the
                        # tt1 x prefetch so that lands first
                        nc.sync.dma_start(out=cc_sb, in_=cc)
                        nc.sync.dma_start(out=ss_sb, in_=ss)
                        nc.sync.dma_start(out=m1_sb, in_=m1)
                        for hh in range(HPC):
                            nc.sync.dma_start(
                                out=wo_sb[:, hh, :],
                                in_=wo[HD * hh : HD * hh + HD, :],
                            )
                    # QT / KT rows (transposed layout); RoPE applied below
                    for w_sb, out_sb in (
                        (wq_sb, qt_sb),
                        (wk_sb, kt_sb),
                    ):
                        for rt in range(HPC):
                            pp = pA.tile([128, 512], F32, tag="pA")
                            for c in range(16):
                                nc.tensor.matmul(
                                    pp,
                                    w_sb[:, c, 128 * rt : 128 * rt + 128],
                                    xc[:, c, :],
                                    start=(c == 0),
                                    stop=(c == 15),
                                )
                            dst = out_sb[:, rt, 512 * tt : 512 * tt + 512]
                            nc.scalar.activation(
                                dst, pp, mybir.ActivationFunctionType.Copy
                            )
                            drain(pool=pST, tag="pST")
                    # RoPE for this 512-token quarter, both heads at once:
                    # issued before the V chains so the DVE work hides under
                    # them and the last quarter's rope never delays attention
                    sl = slice(512 * tt, 512 * tt + 512)
                    for out_sb in (qt_sb, kt_sb):
                        rope_rows = out_sb[0:ROPE, :, sl]
                        swap = ropepool.tile([ROPE, HPC, 512], BF16, tag="rope")
                        nc.sync.dma_start(
                            out=swap[0:32], in_=out_sb[32:64, :, sl]
                        )
                        nc.sync.dma_start(
                            out=swap[32:64], in_=out_sb[0:32, :, sl]
                        )
                        nc.vector.tensor_mul(swap, swap, ss_sb[:, :, sl])
                        nc.vector.tensor_mul(
                            rope_rows, rope_rows, cc_sb[:, :, sl]
                        )
                        nc.vector.tensor_add(rope_rows, rope_rows, swap)

                    # V rows (natural layout), 4 row-groups of 128 tokens.
                    # The last group of the last quarter is deferred into the
                    # attention prologue so its PE work covers the first
                    # instance's exp latency.
                    def v_chain(xc_, tt_, g):
                        pv = pA.tile([128, 512], F32, tag="pA")
                        pvj = pv[:, 0:DQ]
                        for c in range(16):
                            nc.tensor.matmul(
                                pvj,
                                xc_[:, c, 128 * g : 128 * g + 128],
                                wv_sb[:, c, :],
                                start=(c == 0),
                                stop=(c == 15),
                            )
                        nc.vector.tensor_copy(v_sb[:, 4 * tt_ + g, :], pvj)
                        drain(pool=pST, tag="pST")

                    last_tt = tt == NTT - 1
                    for g in range(3 if last_tt else 4):
                        v_chain(xc, tt, g)
                    if last_tt:
                        deferred_v.append(
                            (lambda xc_=xc, tt_=tt: v_chain(xc_, tt_, 3))
                        )


                # ---------- attention (y interleaved via pending queue) --
                # One software pipeline ACROSS all (t, h) instances: produces
                # (ST+exp+mask) run DEPTH slots ahead of consumes (r/AV), so
                # the PE never drains at instance boundaries.
                DEPTH = 3
                av_rp = {}
                pend_r = {}

                def produce(t, h, j):
                    q = j - 4 * t
                    lo = 128 * q if q > 0 else 0
                    st = pST.tile([128, 512], F32, tag="pST")
                    nc.tensor.matmul(
                        st[:, lo:512],
                        kt_sb[:, h, 128 * j : 128 * j + 128],
                        qt_sb[:, h, 512 * t + lo : 512 * t + 512],
                        start=True,
                        stop=True,
                    )
                    et = etpool.tile([128, 512], BF16, tag="et")
                    nc.scalar.activation(
                        et[:, lo:512],
                        st[:, lo:512],
                        mybir.ActivationFunctionType.Exp,
                    )
                    if q >= 0:
                        nc.vector.tensor_mul(
                            et[:, lo : lo + 128],
                            et[:, lo : lo + 128],
                            m1_sb,
                        )
                    return t, h, j, lo, et

                def consume(slot):
                    t, h, j, lo, et = slot
                    njb = 4 * t + 4
                    if j == 0:
                        av_rp[(t, h)] = (
                            pAV.tile([128, 512], F32, tag="pAV",
                                     name=f"av_{b}_{t}_{h}"),
                            pR.tile([128, 512], F32, tag="pR",
                                    name=f"rp_{b}_{t}_{h}"),
                        )
                    av, rp = av_rp[(t, h)]
                    # drain first: fills the PE while exp(j) finishes.
                    # t3 has 32 slots for 16 pendings: drain odd slots only
                    # so fill work lasts the whole instance
                    do = t < NI - 1 or j % 2 == 1
                    if b == B - 1 and t == NI - 1:
                        do = do and len(pending) > 4
                    if do:
                        drain()
                    if j <= 1:
                        # first two blocks keep per-block r-matmuls so the
                        # rp group has exactly one full-width start=True
                        nc.tensor.matmul(
                            rp[:, lo:512],
                            ones,
                            et[:, lo:512],
                            start=(j == 0),
                            stop=False,
                        )
                    elif j % 2 == 0:
                        pend_r[(t, h)] = (lo, et)
                    else:
                        # denominators per block PAIR: one cheap bf16 DVE add
                        # replaces a whole PE matmul pass; ragged diagonal
                        # pairs get a 128-wide strip matmul
                        lo0, et0 = pend_r.pop((t, h))
                        es = espool.tile(
                            [128, 512], BF16, tag="es",
                            name=f"es_{b}_{t}_{h}_{j}",
                        )
                        nc.vector.tensor_add(
                            es[:, lo:512], et0[:, lo:512], et[:, lo:512]
                        )
                        nc.tensor.matmul(
                            rp[:, lo:512],
                            ones,
                            es[:, lo:512],
                            start=False,
                            stop=(j == njb - 1 and lo0 == lo),
                        )
                        if lo0 < lo:
                            nc.tensor.matmul(
                                rp[:, lo0:lo],
                                ones,
                                et0[:, lo0:lo],
                                start=False,
                                stop=(j == njb - 1),
                            )
                    nc.tensor.matmul(
                        av[:, lo:512],
                        v_sb[:, j, HD * h : HD * h + HD],
                        et[:, lo:512],
                        start=(j == 0),
                        stop=(j == njb - 1),
                    )
                    if j == njb - 1:
                        av, rp = av_rp.pop((t, h))
                        rec = recpool.tile([128, 512], F32, tag="rec")
                        nc.vector.reciprocal(rec, rp)
                        nc.vector.tensor_tensor(
                            att_sb[:, h, 512 * t : 512 * t + 512],
                            av,
                            rec,
                            op=mybir.AluOpType.mult,
                        )
                        if h == HPC - 1:
                            pending.extend(
                                (b, t2, dd)
                                for t2 in range(4 * t, 4 * t + 4)
                                for dd in range(4)
                            )

                stream = [
                    (t, h, j)
                    for t in range(NI)
                    for h in range(HPC)
                    for j in range(4 * t + 4)
                ]
                window = []
                for n_, thj in enumerate(stream):
                    window.append(produce(*thj))
                    if n_ == DEPTH - 1 and deferred_v:
                        deferred_v.pop()()
                    if len(window) > DEPTH:
                        consume(window.pop(0))
                for slot in window:
                    consume(slot)
            # final drain (tail of last batch): ST/AV PSUM banks are free
            # here, so rotate across all pools to keep emissions in flight
            rot = [
                (pA, "pA"), (pST, "pST"), (pA, "pA"),
                (pST, "pST"), (pA, "pA"), (pAV, "pAV"),
            ]
            i = 0
            while pending:
                pool, tag = rot[i % len(rot)]
                emit_y(
                    *pending.pop(0), pool=pool, tag=tag,
                    split_dma=len(pending) < 8,
                )
                i += 1
    nc.compile()
    return nc


_NC = None


def _get_nc():
    global _NC
    if _NC is None:
        _NC = build_nc()
    return _NC


def _host_inputs(x, mask, wq, wk, wv, wo):
    import ml_dtypes

    x = np.asarray(x, np.float32)
    wq = np.asarray(wq, np.float32)
    wk = np.asarray(wk, np.float32)
    wv = np.asarray(wv, np.float32)
    wo = np.asarray(wo, np.float32)

    xt = np.ascontiguousarray(x.transpose(0, 2, 1)).astype(ml_dtypes.bfloat16)

    # permute head dims so RoPE pairs are (i, i+32): [evens, odds, pass-through]
    perm128 = np.concatenate(
        [np.arange(0, ROPE, 2), np.arange(1, ROPE, 2), np.arange(ROPE, HD)]
    )
    permD = np.concatenate([h * HD + perm128 for h in range(H)])
    wq_p = (wq * np.float32(1.0 / np.sqrt(HD)))[:, permD]
    wk_p = wk[:, permD]

    # RoPE tables, matching reference fp32 math (dim=64, repeat-2 interleave)
    # cc = [cos; cos], ss = [-sin; +sin] for the (x1=rows 0:32, x2=rows 32:64)
    # pairing: rot = [x1;x2]*cc + [x2;x1]*ss  (duplicated across head axis)
    ts_ = np.arange(0, ROPE, 2, dtype=np.float32)
    inv = (np.float32(10000.0) ** (-ts_ / np.float32(ROPE))).astype(np.float32)
    grid = np.arange(L, dtype=np.float32)[:, None] * inv[None, :]  # [L, 32]
    cc1 = np.empty((ROPE, L), np.float32)
    cc1[0:32] = cc1[32:64] = np.cos(grid).T
    ss1 = np.empty((ROPE, L), np.float32)
    ss1[0:32] = -np.sin(grid).T
    ss1[32:64] = np.sin(grid).T
    cc = np.repeat(cc1[:, None, :], HPC, axis=1).astype(ml_dtypes.bfloat16)
    ss = np.repeat(ss1[:, None, :], HPC, axis=1).astype(ml_dtypes.bfloat16)

    # strip mask for diagonal j-blocks: valid iff (i - 128q) >= j
    jj = np.arange(128)
    m1 = (jj[None, :] >= jj[:, None]).astype(ml_dtypes.bfloat16)

    in_maps = []
    for c in range(NCORES):
        sl = slice(DQ * c, DQ * c + DQ)
        in_maps.append(
            {
                "xt": xt,
                "wq": np.ascontiguousarray(wq_p[:, sl]).astype(
                    ml_dtypes.bfloat16
                ),
                "wk": np.ascontiguousarray(wk_p[:, sl]).astype(
                    ml_dtypes.bfloat16
                ),
                "wv": np.ascontiguousarray(wv[:, sl]).astype(
                    ml_dtypes.bfloat16
                ),
                "wo": np.ascontiguousarray(wo[sl, :]).astype(
                    ml_dtypes.bfloat16
                ),
                "cc": cc,
                "ss": ss,
                "m1": m1,
            }
        )
    return in_maps


def _reference_host(x, mask, wq, wk, wv, wo):
    """Exact-math numpy fallback (used only if the mask is not causal-tril)."""
    Hh, P = H, 64
    xx = np.asarray(x, np.float32)
    Bb, Ll, Dd = xx.shape
    K = Dd // Hh

    def rope(t):  # [b,h,s,d]
        d, s = t.shape[-1], t.shape[-2]
        ts_ = np.arange(0, d, 2, dtype=np.float32)
        inv = np.float32(10000.0) ** (-ts_ / np.float32(d))
        grid = np.arange(s, dtype=np.float32)[:, None] * inv[None, :]
        sin = np.repeat(np.sin(grid), 2, axis=-1)[None, None]
        cos = np.repeat(np.cos(grid), 2, axis=-1)[None, None]
        x1, x2 = t[..., ::2], t[..., 1::2]
        xs = np.stack([-x2, x1], axis=-1).reshape(t.shape)
        return t * cos + xs * sin

    def split(t):
        return t.reshape(Bb, Ll, Hh, K).transpose(0, 2, 1, 3)

    q = split(xx @ np.asarray(wq, np.float32)) / np.sqrt(K)
    q = np.concatenate([rope(q[..., :P]), q[..., P:]], axis=-1)
    k = split(xx @ np.asarray(wk, np.float32))
    k = np.concatenate([rope(k[..., :P]), k[..., P:]], axis=-1)
    v = split(xx @ np.asarray(wv, np.float32))
    s = np.einsum("bhik,bhjk->bhij", q, k)
    s = np.where(np.asarray(mask), s, np.float32(-1e8))
    s -= s.max(axis=-1, keepdims=True)
    e = np.exp(s)
    a = e / e.sum(axis=-1, keepdims=True)
    yy = np.einsum("bhij,bhjv->bhiv", a, v)
    yy = yy.transpose(0, 2, 1, 3).reshape(Bb, Ll, Dd)
    return (yy @ np.asarray(wo, np.float32)).astype(np.float32)


def kernel(**inputs):
    mask_arr = np.asarray(inputs["mask"])
    if not bool((mask_arr[0, 0] == np.tril(np.ones((L, L), bool))).all()):
        return _reference_host(
            inputs["x"], inputs["mask"], inputs["wq"], inputs["wk"],
            inputs["wv"], inputs["wo"],
        )
    nc = _get_nc()
    in_maps = _host_inputs(
        inputs["x"], inputs["mask"], inputs["wq"], inputs["wk"],
        inputs["wv"], inputs["wo"],
    )
    res = run_bass_kernel_spmd(nc, in_maps, core_ids=list(range(NCORES)))
    out = np.zeros((B, L, D), np.float64)
    for c in range(NCORES):
        out += np.asarray(res.results[c]["y"], np.float64)
    return out.astype(np.float32)


# revision 6
# speedup vs baseline: 1.0381x; 1.0046x over previous
"""Multi-head attention (B=2, L=2048, D=2048, H=16, causal + RoPE) on 8 TRN2 cores.

Sharding: tensor-parallel over heads. Core c owns heads {2c, 2c+1}:
  - wq/wk/wv column slices [D, 256], wo row slice [256, D]
  - each core computes a partial output y_c = att_c @ wo_c (full shape, bf16)
  - host reduces: y = sum_c y_c  (the "all-reduce" of the output projection)

v2 design (all-bf16 operands, fp32 PSUM accumulation):
  1. Projections: QT/KT = (w^T x^T) in transposed layout [head_dim, tok],
     V in natural layout [tok, head_dim]; all matmul operands bf16 (1 cyc/row).
     RoPE applied to QT/KT rows 0:64 per head with head-duplicated cc/ss
     tables so each DVE op covers both heads ([64, 2, 512], bf16 2x mode).
  2. Attention per (batch, head): causal, no-max-subtraction softmax.
     Per j-block of 128 keys against a 512-query i-tile:
       ST = K_j^T Q_i   (PE)     -- diagonal blocks narrowed to valid columns
       ET = exp(ST)     (ACT, bf16 out) ; diagonal strip masked via one
                                  [128,128] tril mult (DVE, bf16 2x)
       R  += ones^T ET  (PE)     -- ones stationary is [128,128] so R PSUM
                                    holds denominators broadcast to all
                                    partitions: no transpose bounce needed
       AV += V_j^T ET   (PE)
     Then att[:, h, i-tile] = AV * reciprocal(R) -- two DVE ops, fused
     normalize + PSUM->SBUF copy.
  3. Output: y[tok,:] = sum_h att_h^T wo_h ; both heads accumulate into one
     PSUM bank; PSUM->SBUF copies alternate ACT/DVE into a [128, 4, 512]
     bf16 stage written out as one [128, 2048] DMA per 128-token chunk.
     Emissions are interleaved ("pending" queue) into later attention
     j-blocks and the next batch's projection chains so the PE never idles
     through an emission-only phase.
"""

import glob
import os


def _ensure_env():
    # walrus_driver (neuronx-cc) must be on PATH for client-side NEFF compile.
    if not any("-b16-bazel-" in p for p in os.environ.get("PATH", "").split(":")):
        cands = sorted(glob.glob("/nix/store/*-b16-bazel-*/bin"))
        for c in cands:
            if os.path.exists(os.path.join(c, "neuronx-cc")) or glob.glob(
                os.path.join(c, "*walrus*")
            ):
                os.environ["PATH"] = c + ":" + os.environ["PATH"]
                break
        else:
            if cands:
                os.environ["PATH"] = cands[-1] + ":" + os.environ["PATH"]


_ensure_env()
os.environ.setdefault("JAX_COMPILATION_CACHE_DIR", "/tmp/jax_comp_cache")
os.environ.setdefault("JAX_PERSISTENT_CACHE_MIN_COMPILE_TIME_SECS", "1")
os.environ.setdefault("JAX_PERSISTENT_CACHE_MIN_ENTRY_SIZE_BYTES", "0")

import numpy as np  # noqa: E402

import concourse.bass as bass  # noqa: E402
import concourse.mybir as mybir  # noqa: E402
import concourse.tile as tile  # noqa: E402
from concourse import bacc  # noqa: E402
from concourse.bass_utils import run_bass_kernel_spmd  # noqa: E402

NCORES = 8
B, L, D = 2, 2048, 2048
H = 16
HD = 128            # head dim
HPC = H // NCORES   # heads per core
DQ = HPC * HD       # 256: per-core projection width
ROPE = 64           # RoPE dims per head
F32 = mybir.dt.float32
BF16 = mybir.dt.bfloat16

NTT = L // 512      # 4 token tiles (512) for projections
NI = L // 512       # 4 i-tiles per attention instance
NJ = L // 128       # 16 j-blocks


def build_nc():
    nc = bacc.Bacc(
        "TRN2", target_bir_lowering=False, debug=False, num_devices=NCORES
    )
    xt = nc.dram_tensor("xt", [B, D, L], BF16, kind="ExternalInput").ap()
    wq = nc.dram_tensor("wq", [D, DQ], BF16, kind="ExternalInput").ap()
    wk = nc.dram_tensor("wk", [D, DQ], BF16, kind="ExternalInput").ap()
    wv = nc.dram_tensor("wv", [D, DQ], BF16, kind="ExternalInput").ap()
    wo = nc.dram_tensor("wo", [DQ, D], BF16, kind="ExternalInput").ap()
    # cc rows = [cos;cos], ss rows = [-sin;+sin], duplicated per head so one
    # DVE op can cover both heads' rope rows: [ROPE, HPC, L]
    cc = nc.dram_tensor("cc", [ROPE, HPC, L], BF16, kind="ExternalInput").ap()
    ss = nc.dram_tensor("ss", [ROPE, HPC, L], BF16, kind="ExternalInput").ap()
    # strip mask for diagonal j-blocks: m1[j, x] = 1.0 if x >= j else 0
    m1 = nc.dram_tensor("m1", [128, 128], BF16, kind="ExternalInput").ap()
    y = nc.dram_tensor("y", [B, L, D], BF16, kind="ExternalOutput").ap()

    with tile.TileContext(nc) as tc:
        with (
            tc.tile_pool(name="consts", bufs=1) as consts,
            tc.tile_pool(name="wpool", bufs=1) as wpool,
            tc.tile_pool(name="qkv", bufs=1) as qkv,
            tc.tile_pool(name="xc", bufs=2) as xcpool,
            tc.tile_pool(name="et", bufs=6) as etpool,
            tc.tile_pool(name="es", bufs=3) as espool,
            tc.tile_pool(name="rope", bufs=4) as ropepool,
            tc.tile_pool(name="rec", bufs=2) as recpool,
            tc.tile_pool(name="ysb", bufs=4) as ypool,
            tc.tile_pool(name="pA", bufs=2, space="PSUM") as pA,
            tc.tile_pool(name="pST", bufs=3, space="PSUM") as pST,
            tc.tile_pool(name="pAV", bufs=2, space="PSUM") as pAV,
            tc.tile_pool(name="pR", bufs=1, space="PSUM") as pR,
        ):
            # ---- constants / weights ----
            ones = consts.tile([128, 128], BF16)
            nc.vector.memset(ones, 1.0)
            # warm the PE p-state during the startup DMA wait: tiny matmuls
            # keep the tensor engine continuously busy so the first real
            # chains run at full clock (ramp needs 3us of busy)
            pwarm = pR.tile([128, 64], F32, tag="pR", name="pwarm")
            for _ in range(60):
                nc.tensor.matmul(
                    pwarm[0:1, :], ones[:, 0:1], ones[:, 0:64],
                    start=True, stop=True,
                )
            m1_sb = consts.tile([128, 128], BF16)
            cc_sb = consts.tile([ROPE, HPC, L], BF16)
            ss_sb = consts.tile([ROPE, HPC, L], BF16)
            xc_pre = xcpool.tile([128, 16, 512], BF16, tag="xc")
            wq_sb = wpool.tile([128, 16, DQ], BF16)
            wk_sb = wpool.tile([128, 16, DQ], BF16)
            wv_sb = wpool.tile([128, 16, DQ], BF16)
            wo_sb = wpool.tile([128, HPC, D], BF16)

            def _w_oct(w_dram, w_sb, oct_):
                nc.sync.dma_start(
                    out=w_sb[:, 2 * oct_ : 2 * oct_ + 2, :],
                    in_=w_dram[256 * oct_ : 256 * oct_ + 256, :].rearrange(
                        "(c p) o -> p c o", p=128
                    ),
                )

            # DMA order = first-use order: the first q-projection chain needs
            # xc oct_c + wq oct_c in sequence, so interleave them; wk/wv feed
            # the following chains; rope tables before the first rope; wo and
            # the mask strip are only needed tens of us in.
            for oct_ in range(8):
                nc.sync.dma_start(
                    out=xc_pre[:, 2 * oct_ : 2 * oct_ + 2, :],
                    in_=xt[0, 256 * oct_ : 256 * oct_ + 256, 0:512].rearrange(
                        "(c p) t -> p c t", p=128
                    ),
                )
                _w_oct(wq, wq_sb, oct_)
            for w_dram, w_sb in ((wk, wk_sb), (wv, wv_sb)):
                for qq in range(4):
                    nc.sync.dma_start(
                        out=w_sb[:, 4 * qq : 4 * qq + 4, :],
                        in_=w_dram[
                            512 * qq : 512 * qq + 512, :
                        ].rearrange("(c p) o -> p c o", p=128),
                    )

            qt_sb = qkv.tile([128, HPC, L], BF16)   # [d, h, tok]
            kt_sb = qkv.tile([128, HPC, L], BF16)
            v_sb = qkv.tile([128, NJ, DQ], BF16)    # [tok_in_blk, jblk, hd]
            att_sb = qkv.tile([128, HPC, L], BF16)  # [hd, h, tok] normalized

            # ---- y emission machinery ----
            pending = []          # (b, t2) chunks with dd sub-emissions
            ystage = {}           # t2 -> stage tile
            ecount = [0]          # emission counter for ACT/DVE alternation

            def emit_y(b_, t2, dd, pool=None, tag="pA", eng="alt", split_dma=False):
                pool = pool or pA
                p = pool.tile([128, 512], F32, tag=tag, name=f"yp_{b_}_{t2}_{dd}")
                nc.tensor.matmul(
                    p,
                    att_sb[:, 0, 128 * t2 : 128 * t2 + 128],
                    wo_sb[:, 0, 512 * dd : 512 * dd + 512],
                    start=True,
                    stop=False,
                )
                nc.tensor.matmul(
                    p,
                    att_sb[:, 1, 128 * t2 : 128 * t2 + 128],
                    wo_sb[:, 1, 512 * dd : 512 * dd + 512],
                    start=False,
                    stop=True,
                )
                if dd == 0:
                    ystage[t2] = ypool.tile(
                        [128, 4, 512], BF16, tag="ysb", name=f"yst_{b_}_{t2}"
                    )
                dst = ystage[t2][:, dd, :]
                # ACT is the exp critical path during attention: route those
                # drains' copies to DVE; alternate engines elsewhere
                if eng == "dve" or (eng == "alt" and ecount[0] % 2 == 1):
                    nc.vector.tensor_copy(dst, p)
                else:
                    nc.scalar.activation(
                        dst, p, mybir.ActivationFunctionType.Copy
                    )
                ecount[0] += 1
                if split_dma:
                    qeng = (nc.gpsimd, nc.sync, nc.gpsimd, nc.sync)[dd]
                    qeng.dma_start(
                        out=y[
                            b_,
                            128 * t2 : 128 * t2 + 128,
                            512 * dd : 512 * dd + 512,
                        ],
                        in_=dst,
                    )
                    if dd == 3:
                        ystage.pop(t2)
                elif dd == 3:
                    nc.gpsimd.dma_start(
                        out=y[b_, 128 * t2 : 128 * t2 + 128, :],
                        in_=ystage.pop(t2),
                    )

            def drain(n=1, eng="alt", pool=None, tag="pA"):
                for _ in range(n):
                    if pending:
                        emit_y(*pending.pop(0), eng=eng, pool=pool, tag=tag)

            xc_tiles = {(0, 0): xc_pre}

            def xc_load(b_, tt_):
                # prefetch: issue the x-chunk DMAs one tile ahead of use
                if (b_, tt_) in xc_tiles or b_ >= B:
                    return
                xct = xcpool.tile(
                    [128, 16, 512], BF16, tag="xc", name=f"xc_{b_}_{tt_}"
                )
                for qq in range(4):
                    nc.sync.dma_start(
                        out=xct[:, 4 * qq : 4 * qq + 4, :],
                        in_=xt[
                            b_,
                            512 * qq : 512 * qq + 512,
                            512 * tt_ : 512 * tt_ + 512,
                        ].rearrange("(c p) t -> p c t", p=128),
                    )
                xc_tiles[(b_, tt_)] = xct

            deferred_v = []

            for b in range(B):
                # ---------- projections ----------
                for tt in range(NTT):
                    xc_load(b, tt)
                    xc = xc_tiles.pop((b, tt))
                    if tt + 1 < NTT:
                        xc_load(b, tt + 1)
                    else:
                        xc_load(b + 1, 0)
                    if b == 0 and tt == 0:
                        # tables/masks/wo are needed late; issue after the
                        # tt1 x prefetch so that lands first
                        nc.sync.dma_start(out=cc_sb, in_=cc)
                        nc.sync.dma_start(out=ss_sb, in_=ss)
                        nc.sync.dma_start(out=m1_sb, in_=m1)
                        for hh in range(HPC):
                            nc.sync.dma_start(
                                out=wo_sb[:, hh, :],
                                in_=wo[HD * hh : HD * hh + HD, :],
                            )
                    # QT / KT rows (transposed layout); RoPE applied below
                    for w_sb, out_sb in (
                        (wq_sb, qt_sb),
                        (wk_sb, kt_sb),
                    ):
                        for rt in range(HPC):
                            pp = pA.tile([128, 512], F32, tag="pA")
                            for c in range(16):
                                nc.tensor.matmul(
                                    pp,
                                    w_sb[:, c, 128 * rt : 128 * rt + 128],
                                    xc[:, c, :],
                                    start=(c == 0),
                                    stop=(c == 15),
                                )
                            dst = out_sb[:, rt, 512 * tt : 512 * tt + 512]
                            nc.scalar.activation(
                                dst, pp, mybir.ActivationFunctionType.Copy
                            )
                            drain(pool=pST, tag="pST")
                    # RoPE for this 512-token quarter, both heads at once:
                    # issued before the V chains so the DVE work hides under
                    # them and the last quarter's rope never delays attention
                    sl = slice(512 * tt, 512 * tt + 512)
                    for out_sb in (qt_sb, kt_sb):
                        rope_rows = out_sb[0:ROPE, :, sl]
                        swap = ropepool.tile([ROPE, HPC, 512], BF16, tag="rope")
                        nc.sync.dma_start(
                            out=swap[0:32], in_=out_sb[32:64, :, sl]
                        )
                        nc.sync.dma_start(
                            out=swap[32:64], in_=out_sb[0:32, :, sl]
                        )
                        nc.vector.tensor_mul(swap, swap, ss_sb[:, :, sl])
                        nc.vector.tensor_mul(
                            rope_rows, rope_rows, cc_sb[:, :, sl]
                        )
                        nc.vector.tensor_add(rope_rows, rope_rows, swap)

                    # V rows (natural layout), 4 row-groups of 128 tokens.
                    # The last group of the last quarter is deferred into the
                    # attention prologue so its PE work covers the first
                    # instance's exp latency.
                    def v_chain(xc_, tt_, g):
                        pv = pA.tile([128, 512], F32, tag="pA")
                        pvj = pv[:, 0:DQ]
                        for c in range(16):
                            nc.tensor.matmul(
                                pvj,
                                xc_[:, c, 128 * g : 128 * g + 128],
                                wv_sb[:, c, :],
                                start=(c == 0),
                                stop=(c == 15),
                            )
                        nc.vector.tensor_copy(v_sb[:, 4 * tt_ + g, :], pvj)
                        drain(pool=pST, tag="pST")

                    last_tt = tt == NTT - 1
                    for g in range(3 if last_tt else 4):
                        v_chain(xc, tt, g)
                    if last_tt:
                        deferred_v.append(
                            (lambda xc_=xc, tt_=tt: v_chain(xc_, tt_, 3))
                        )


                # ---------- attention (y interleaved via pending queue) --
                # One software pipeline ACROSS all (t, h) instances: produces
                # (ST+exp+mask) run DEPTH slots ahead of consumes (r/AV), so
                # the PE never drains at instance boundaries.
                DEPTH = 3
                av_rp = {}
                pend_r = {}

                def produce(t, h, j):
                    q = j - 4 * t
                    lo = 128 * q if q > 0 else 0
                    st = pST.tile([128, 512], F32, tag="pST")
                    nc.tensor.matmul(
                        st[:, lo:512],
                        kt_sb[:, h, 128 * j : 128 * j + 128],
                        qt_sb[:, h, 512 * t + lo : 512 * t + 512],
                        start=True,
                        stop=True,
                    )
                    et = etpool.tile([128, 512], BF16, tag="et")
                    nc.scalar.activation(
                        et[:, lo:512],
                        st[:, lo:512],
                        mybir.ActivationFunctionType.Exp,
                    )
                    if q >= 0:
                        nc.vector.tensor_mul(
                            et[:, lo : lo + 128],
                            et[:, lo : lo + 128],
                            m1_sb,
                        )
                    return t, h, j, lo, et

                def consume(slot):
                    t, h, j, lo, et = slot
                    njb = 4 * t + 4
                    if j == 0:
                        av_rp[(t, h)] = (
                            pAV.tile([128, 512], F32, tag="pAV",
                                     name=f"av_{b}_{t}_{h}"),
                            pR.tile([128, 512], F32, tag="pR",
                                    name=f"rp_{b}_{t}_{h}"),
                        )
                    av, rp = av_rp[(t, h)]
                    # drain first: fills the PE while exp(j) finishes.
                    # t3 has 32 slots for 16 pendings: drain odd slots only
                    # so fill work lasts the whole instance
                    do = t < NI - 1 or j % 2 == 1
                    if b == B - 1 and t == NI - 1:
                        do = do and len(pending) > 4
                    if do:
                        drain()
                    if t == 0 and j <= 1:
                        # t0's ragged first pair keeps per-block r-matmuls so
                        # the rp group has exactly one full-width start=True;
                        # for t>=1 the first pair is full-width on both blocks
                        # and pairs safely (single start via the pair matmul)
                        nc.tensor.matmul(
                            rp[:, lo:512],
                            ones,
                            et[:, lo:512],
                            start=(j == 0),
                            stop=False,
                        )
                    elif j % 2 == 0:
                        pend_r[(t, h)] = (lo, et)
                    else:
                        # denominators per block PAIR: one cheap bf16 DVE add
                        # replaces a whole PE matmul pass; ragged diagonal
                        # pairs get a 128-wide strip matmul
                        lo0, et0 = pend_r.pop((t, h))
                        es = espool.tile(
                            [128, 512], BF16, tag="es",
                            name=f"es_{b}_{t}_{h}_{j}",
                        )
                        nc.vector.tensor_add(
                            es[:, lo:512], et0[:, lo:512], et[:, lo:512]
                        )
                        nc.tensor.matmul(
                            rp[:, lo:512],
                            ones,
                            es[:, lo:512],
                            start=(j == 1),
                            stop=(j == njb - 1 and lo0 == lo),
                        )
                        if lo0 < lo:
                            nc.tensor.matmul(
                                rp[:, lo0:lo],
                                ones,
                                et0[:, lo0:lo],
                                start=False,
                                stop=(j == njb - 1),
                            )
                    nc.tensor.matmul(
                        av[:, lo:512],
                        v_sb[:, j, HD * h : HD * h + HD],
                        et[:, lo:512],
                        start=(j == 0),
                        stop=(j == njb - 1),
                    )
                    if j == njb - 1:
                        av, rp = av_rp.pop((t, h))
                        rec = recpool.tile([128, 512], F32, tag="rec")
                        nc.vector.reciprocal(rec, rp)
                        nc.vector.tensor_tensor(
                            att_sb[:, h, 512 * t : 512 * t + 512],
                            av,
                            rec,
                            op=mybir.AluOpType.mult,
                        )
                        if h == HPC - 1:
                            pending.extend(
                                (b, t2, dd)
                                for t2 in range(4 * t, 4 * t + 4)
                                for dd in range(4)
                            )

                stream = [
                    (t, h, j)
                    for t in range(NI)
                    for h in range(HPC)
                    for j in range(4 * t + 4)
                ]
                window = []
                for n_, thj in enumerate(stream):
                    window.append(produce(*thj))
                    if n_ == DEPTH - 1 and deferred_v:
                        deferred_v.pop()()
                    if len(window) > DEPTH:
                        consume(window.pop(0))
                for slot in window:
                    consume(slot)
            # final drain (tail of last batch): ST/AV PSUM banks are free
            # here, so rotate across all pools to keep emissions in flight
            rot = [
                (pA, "pA"), (pST, "pST"), (pA, "pA"),
                (pST, "pST"), (pA, "pA"), (pAV, "pAV"),
            ]
            i = 0
            while pending:
                pool, tag = rot[i % len(rot)]
                emit_y(
                    *pending.pop(0), pool=pool, tag=tag,
                    split_dma=len(pending) < 8,
                )
                i += 1
    nc.compile()
    return nc


_NC = None


def _get_nc():
    global _NC
    if _NC is None:
        _NC = build_nc()
    return _NC


def _host_inputs(x, mask, wq, wk, wv, wo):
    import ml_dtypes

    x = np.asarray(x, np.float32)
    wq = np.asarray(wq, np.float32)
    wk = np.asarray(wk, np.float32)
    wv = np.asarray(wv, np.float32)
    wo = np.asarray(wo, np.float32)

    xt = np.ascontiguousarray(x.transpose(0, 2, 1)).astype(ml_dtypes.bfloat16)

    # permute head dims so RoPE pairs are (i, i+32): [evens, odds, pass-through]
    perm128 = np.concatenate(
        [np.arange(0, ROPE, 2), np.arange(1, ROPE, 2), np.arange(ROPE, HD)]
    )
    permD = np.concatenate([h * HD + perm128 for h in range(H)])
    wq_p = (wq * np.float32(1.0 / np.sqrt(HD)))[:, permD]
    wk_p = wk[:, permD]

    # RoPE tables, matching reference fp32 math (dim=64, repeat-2 interleave)
    # cc = [cos; cos], ss = [-sin; +sin] for the (x1=rows 0:32, x2=rows 32:64)
    # pairing: rot = [x1;x2]*cc + [x2;x1]*ss  (duplicated across head axis)
    ts_ = np.arange(0, ROPE, 2, dtype=np.float32)
    inv = (np.float32(10000.0) ** (-ts_ / np.float32(ROPE))).astype(np.float32)
    grid = np.arange(L, dtype=np.float32)[:, None] * inv[None, :]  # [L, 32]
    cc1 = np.empty((ROPE, L), np.float32)
    cc1[0:32] = cc1[32:64] = np.cos(grid).T
    ss1 = np.empty((ROPE, L), np.float32)
    ss1[0:32] = -np.sin(grid).T
    ss1[32:64] = np.sin(grid).T
    cc = np.repeat(cc1[:, None, :], HPC, axis=1).astype(ml_dtypes.bfloat16)
    ss = np.repeat(ss1[:, None, :], HPC, axis=1).astype(ml_dtypes.bfloat16)

    # strip mask for diagonal j-blocks: valid iff (i - 128q) >= j
    jj = np.arange(128)
    m1 = (jj[None, :] >= jj[:, None]).astype(ml_dtypes.bfloat16)

    in_maps = []
    for c in range(NCORES):
        sl = slice(DQ * c, DQ * c + DQ)
        in_maps.append(
            {
                "xt": xt,
                "wq": np.ascontiguousarray(wq_p[:, sl]).astype(
                    ml_dtypes.bfloat16
                ),
                "wk": np.ascontiguousarray(wk_p[:, sl]).astype(
                    ml_dtypes.bfloat16
                ),
                "wv": np.ascontiguousarray(wv[:, sl]).astype(
                    ml_dtypes.bfloat16
                ),
                "wo": np.ascontiguousarray(wo[sl, :]).astype(
                    ml_dtypes.bfloat16
                ),
                "cc": cc,
                "ss": ss,
                "m1": m1,
            }
        )
    return in_maps


def _reference_host(x, mask, wq, wk, wv, wo):
    """Exact-math numpy fallback (used only if the mask is not causal-tril)."""
    Hh, P = H, 64
    xx = np.asarray(x, np.float32)
    Bb, Ll, Dd = xx.shape
    K = Dd // Hh

    def rope(t):  # [b,h,s,d]
        d, s = t.shape[-1], t.shape[-2]
        ts_ = np.arange(0, d, 2, dtype=np.float32)
        inv = np.float32(10000.0) ** (-ts_ / np.float32(d))
        grid = np.arange(s, dtype=np.float32)[:, None] * inv[None, :]
        sin = np.repeat(np.sin(grid), 2, axis=-1)[None, None]
        cos = np.repeat(np.cos(grid), 2, axis=-1)[None, None]
        x1, x2 = t[..., ::2], t[..., 1::2]
        xs = np.stack([-x2, x1], axis=-1).reshape(t.shape)
        return t * cos + xs * sin

    def split(t):
        return t.reshape(Bb, Ll, Hh, K).transpose(0, 2, 1, 3)

    q = split(xx @ np.asarray(wq, np.float32)) / np.sqrt(K)
    q = np.concatenate([rope(q[..., :P]), q[..., P:]], axis=-1)
    k = split(xx @ np.asarray(wk, np.float32))
    k = np.concatenate([rope(k[..., :P]), k[..., P:]], axis=-1)
    v = split(xx @ np.asarray(wv, np.float32))
    s = np.einsum("bhik,bhjk->bhij", q, k)
    s = np.where(np.asarray(mask), s, np.float32(-1e8))
    s -= s.max(axis=-1, keepdims=True)
    e = np.exp(s)
    a = e / e.sum(axis=-1, keepdims=True)
    yy = np.einsum("bhij,bhjv->bhiv", a, v)
    yy = yy.transpose(0, 2, 1, 3).reshape(Bb, Ll, Dd)
    return (yy @ np.asarray(wo, np.float32)).astype(np.float32)


def kernel(**inputs):
    mask_arr = np.asarray(inputs["mask"])
    if not bool((mask_arr[0, 0] == np.tril(np.ones((L, L), bool))).all()):
        return _reference_host(
            inputs["x"], inputs["mask"], inputs["wq"], inputs["wk"],
            inputs["wv"], inputs["wo"],
        )
    nc = _get_nc()
    in_maps = _host_inputs(
        inputs["x"], inputs["mask"], inputs["wq"], inputs["wk"],
        inputs["wv"], inputs["wo"],
    )
    res = run_bass_kernel_spmd(nc, in_maps, core_ids=list(range(NCORES)))
    out = np.zeros((B, L, D), np.float64)
    for c in range(NCORES):
        out += np.asarray(res.results[c]["y"], np.float64)
    return out.astype(np.float32)
